# revision 29
# baseline (speedup 1.0000x reference)
"""Trainium2 Bass kernel: batched dense attention.

Full inputs: queries/keys/values [16, 2048, 64] fp32.
Shards batch dim across 8 NeuronCores (2 batches per core).

Per-core algorithm (batches A, B local):
  S^T[j, q] = K[j, :] . Q[q, :]           (PE, fp16 operands, fp32 PSUM)
  P^T = exp(S^T / 8)                       (ACT, PSUM->SBUF fp16, scale fused)
  O'^T[d', q] = sum_j V'[j, d'] P^T[j, q]  (PE; V' = [V | ones] so row 64 = softmax sums)
  O[q, :] = O'^T[0:64, q].T / O'^T[64, q]  (PE transpose + DVE reciprocal/mult)

Layout tricks:
  - Q^T / K^T built by PE transposes of natural tiles, batch-PAIRED so batch A
    lands on SBUF partitions 0-63 and batch B on 64-127.  QK^T matmuls for the
    two batches then row-pack on the PE (disjoint row groups, concurrent).
  - S^T PSUM tiles are grouped 3-wide ([128, 1536]) so each ACT exp instruction
    amortizes its ~350-cycle overhead over 1536 elements.
"""

import sys
for _p in ("/opt/trn_rl_repo", "/root/.axon_site/_ro/trn_rl_repo"):
    if _p not in sys.path:
        sys.path.insert(0, _p)

import numpy as np

import concourse.bass as bass
import concourse.mybir as mybir
from concourse import bacc
from concourse.tile import TileContext
from concourse.masks import make_identity

F32 = mybir.dt.float32
F16 = mybir.dt.float16
P = 128

N_CORES = 8
B_FULL, N, D = 16, 2048, 64
B_LOC = B_FULL // N_CORES          # 2 batches per core
NT = N // P                        # 16 tiles of 128 along q and j
QB = 512                           # q-block (PSUM bank width in fp32)
NQB = N // QB                      # 4 q-blocks
GROUP = 3                          # S^T stream tiles per ACT exp instruction
N_STREAM = NQB * 2 * NT            # 128 S^T stream tiles per core
N_GROUPS = (N_STREAM + GROUP - 1) // GROUP

_nc_cache = None


def build():
    nc = bacc.Bacc(None, target_bir_lowering=False)
    q_hbm = nc.dram_tensor("queries", [B_LOC, N, D], F32, kind="ExternalInput")
    k_hbm = nc.dram_tensor("keys", [B_LOC, N, D], F32, kind="ExternalInput")
    v_hbm = nc.dram_tensor("values", [B_LOC, N, D], F32, kind="ExternalInput")
    o_hbm = nc.dram_tensor("out", [B_LOC, N, D], F32, kind="ExternalOutput")

    with TileContext(nc) as tc:
        with (
            tc.tile_pool(name="cst", bufs=1) as cst,
            tc.tile_pool(name="stage", bufs=2) as stage,
            tc.tile_pool(name="persist", bufs=1) as persist,
            tc.tile_pool(name="pt", bufs=18) as ptp,
            tc.tile_pool(name="otp", bufs=2) as otp,
            tc.tile_pool(name="ost", bufs=2) as ostp,
            tc.tile_pool(name="st", bufs=2, space="PSUM") as stp,
            tc.tile_pool(name="pv", bufs=2, space="PSUM") as pvp,
        ):
            ident = cst.tile([P, P], F16)
            make_identity(nc, ident)

            # ---- persistent SBUF buffers ----
            # Q^T / K^T, batch-paired: rows 0-63 batch A (d), 64-127 batch B.
            qt = persist.tile([P, N], F16, tag="qt")
            kt = persist.tile([P, N], F16, tag="kt")
            # V' = [V | ones]: [128 j, b, jt, 65] fp16
            v16 = persist.tile([P, B_LOC, NT, D + 1], F16, tag="v16")

            # ---- prep: load, cast, transpose ----
            # staging layout [128, t, (b d)]: per partition q, tile t, batch-major d
            q_res = [q_hbm[b, :, :].rearrange("(t p) d -> p t d", p=P)
                     for b in range(B_LOC)]
            k_res = [k_hbm[b, :, :].rearrange("(t p) d -> p t d", p=P)
                     for b in range(B_LOC)]

            # Loads: each dma_start costs ~650ns serial issue on the SP HWDGE
            # ring and the data phases serialize across the 16 SDMA engines, so
            # order chunks by when the pipeline needs them: K/Q tiles 0-3
            # first (first matmuls), then 4-7, then the rest, V before Q 8-15.
            st32s, st16s = {}, {}
            for name in ("k", "q"):
                st32s[name] = stage.tile([P, NT, B_LOC, D], F32, tag=f"{name}s32",
                                         name=f"{name}s32")
                st16s[name] = stage.tile([P, NT, B_LOC, D], F16, tag=f"{name}s16",
                                         name=f"{name}s16")
            vs32 = stage.tile([P, B_LOC, NT, D], F32, tag="vs32")

            def load_chunk(name, t0, t1, eng=None):
                re_aps = k_res if name == "k" else q_res
                cs = slice(t0, t1)
                for b in range(B_LOC):
                    (eng or nc.sync).dma_start(st32s[name][:, cs, b, :],
                                               re_aps[b][:, cs, :])
                nc.vector.tensor_copy(st16s[name][:, cs], st32s[name][:, cs])

            load_chunk("k", 0, 4)
            load_chunk("q", 0, 4)
            load_chunk("k", 4, 8)
            load_chunk("q", 4, 8)
            load_chunk("k", 8, NT)
            # V: [128, b, t, d] staging -> cast into v16[:, :, :, :64], ones col
            for b in range(B_LOC):
                v_re = v_hbm[b, :, :].rearrange("(t p) d -> p t d", p=P)
                nc.sync.dma_start(vs32[:, b], v_re)
            nc.vector.tensor_copy(v16[:, :, :, 0:D], vs32[:])
            nc.vector.memset(v16[:, :, :, D:D + 1], 1.0)
            load_chunk("q", 8, NT)

            def prep_transpose(name, t, dst):
                """PE-transpose natural tile t of q/k into dst[:, t*128...]."""
                tp_ps = pvp.tile([P, P], F16, tag="pv", name=f"tp_{name}{t}")
                nc.tensor.transpose(tp_ps[:], st16s[name][:, t], ident[:])
                nc.vector.tensor_copy(dst[:, t * P:(t + 1) * P], tp_ps[:])

            # K^T tiles j0-1 now; the rest are emitted lazily inside the qb0
            # loop (pool allocations are FIFO in emission order, so a transpose
            # emitted before its staging data lands would stall the slot ring).
            for t in range(2):
                prep_transpose("k", t, kt)

            # ---- main loop ----
            # stream of S^T tiles: s = qb*32 + 2*j + b
            st_tiles = {}      # group -> psum tile
            pt_tiles = {}      # group -> sbuf fp16 tile
            o_ps = {}          # b -> current PV accumulation psum tile

            # groups never span a q-block boundary: per qb there are 2*NT=32
            # stream tiles -> 10 groups of 3 + 1 group of 2.
            SPQ = 2 * NT                       # stream tiles per q-block
            GPQ = (SPQ + GROUP - 1) // GROUP   # groups per q-block

            def group_of(s):
                qb_, sl = divmod(s, SPQ)
                return qb_ * GPQ + sl // GROUP, sl % GROUP

            def tiles_in_group(g):
                return min(GROUP, SPQ - (g % GPQ) * GROUP)

            def maybe_exp(g):
                """emit exp for group g once all its stream tiles are written"""
                n_in_g = tiles_in_group(g)
                pt_t = ptp.tile([P, GROUP * QB], F16, tag="pt", name=f"ptg{g}")
                nc.scalar.activation(
                    pt_t[:, :n_in_g * QB], st_tiles[g][:, :n_in_g * QB],
                    mybir.ActivationFunctionType.Exp, scale=0.125,
                )
                pt_tiles[g] = pt_t

            # Q^T tiles for q-block 0
            QTPB = QB // P  # q-tiles per q-block
            for t in range(QTPB):
                prep_transpose("q", t, qt)

            for qb in range(NQB):
                qs = slice(qb * QB, (qb + 1) * QB)
                for j in range(NT):
                    if qb == 0 and j + 2 < NT:
                        prep_transpose("k", j + 2, kt)
                    for b in range(B_LOC):
                        s = qb * SPQ + 2 * j + b
                        g, slot = group_of(s)
                        if slot == 0:
                            st_tiles[g] = stp.tile([P, GROUP * QB], F32, tag="st", name=f"stg{g}")
                        rows = slice(b * D, (b + 1) * D)
                        nc.tensor.matmul(
                            st_tiles[g][:, slot * QB:(slot + 1) * QB],
                            kt[rows, j * P:(j + 1) * P],
                            qt[rows, qs],
                            start=True, stop=True,
                        )
                        if slot == tiles_in_group(g) - 1:
                            maybe_exp(g)
                # next q-block's Q^T transposes: emitted before the PV chains
                # claim the pv PSUM slots, so they slip into PE gaps mid-block
                if qb + 1 < NQB:
                    for t in range((qb + 1) * QTPB, (qb + 2) * QTPB):
                        prep_transpose("q", t, qt)
                for b in range(B_LOC):
                    o_ps[b] = pvp.tile([D + 1, QB], F32, tag="pv", name=f"opv{b}")
                # PV: the two batch chains interleave per j so neither waits
                # for the other to fully drain.
                for j in range(NT):
                    for b in range(B_LOC):
                        s = qb * SPQ + 2 * j + b
                        g, slot = group_of(s)
                        nc.tensor.matmul(
                            o_ps[b][:],
                            v16[:, b, j, :],
                            pt_tiles[g][:, slot * QB:(slot + 1) * QB],
                            start=(j == 0), stop=(j == NT - 1),
                            skip_group_check=True,
                        )
                for b in range(B_LOC):
                    # drain O'^T -> fp16 staging, transpose per q-tile, divide, store
                    ot_sb = otp.tile([D + 1, QB], F16, tag="ot", name=f"ot{b}")
                    nc.vector.tensor_copy(ot_sb[:], o_ps[b][:])
                    o_out = ostp.tile([P, NQB, D], F32, tag="oo", name=f"oo{b}")
                    for t in range(QB // P):
                        tp_ps = pvp.tile([P, D + 1], F16, tag="pv")
                        nc.tensor.transpose(
                            tp_ps[:], ot_sb[:, t * P:(t + 1) * P],
                            ident[:D + 1, :D + 1],
                        )
                        recip = ostp.tile([P, 1], F32, tag="recip")
                        nc.vector.reciprocal(recip[:], tp_ps[:, D:D + 1])
                        nc.vector.tensor_scalar(
                            o_out[:, t, :], tp_ps[:, 0:D], recip[:], None,
                            mybir.AluOpType.mult,
                        )
                    o_dst = o_hbm[b, qs, :].rearrange("(t p) d -> p t d", p=P)
                    nc.sync.dma_start(o_dst, o_out[:])

    nc.compile()
    return nc


def get_nc():
    global _nc_cache
    if _nc_cache is None:
        _nc_cache = build()
    return _nc_cache


def kernel(queries: np.ndarray, keys: np.ndarray, values: np.ndarray) -> np.ndarray:
    from concourse.bass_utils import run_bass_kernel_spmd

    queries = np.ascontiguousarray(np.asarray(queries, dtype=np.float32))
    keys = np.ascontiguousarray(np.asarray(keys, dtype=np.float32))
    values = np.ascontiguousarray(np.asarray(values, dtype=np.float32))

    nc = get_nc()
    in_maps = []
    for c in range(N_CORES):
        sl = slice(c * B_LOC, (c + 1) * B_LOC)
        in_maps.append({
            "queries": queries[sl],
            "keys": keys[sl],
            "values": values[sl],
        })
    res = run_bass_kernel_spmd(nc, in_maps, core_ids=list(range(N_CORES)))
    return np.concatenate([r["out"] for r in res.results], axis=0)


if __name__ == "__main__":
    rng = np.random.default_rng(0)
    q = rng.standard_normal((B_FULL, N, D), dtype=np.float32)
    k = rng.standard_normal((B_FULL, N, D), dtype=np.float32)
    v = rng.standard_normal((B_FULL, N, D), dtype=np.float32)
    o = kernel(queries=q, keys=k, values=v)
    s = q @ k.transpose(0, 2, 1) / np.sqrt(D)
    w = np.exp(s - s.max(-1, keepdims=True))
    w /= w.sum(-1, keepdims=True)
    ref = w @ v
    err = np.abs(o - ref).max() / np.abs(ref).max()
    print("rel err:", err)


# revision 41
# speedup vs baseline: 1.0189x; 1.0189x over previous
"""Trainium2 Bass kernel: batched dense attention.

Full inputs: queries/keys/values [16, 2048, 64] fp32.
Shards batch dim across 8 NeuronCores (2 batches per core).

Per-core algorithm (batches A, B local):
  S^T[j, q] = K[j, :] . Q[q, :]           (PE, fp16 operands, fp32 PSUM)
  P^T = exp(S^T / 8)                       (ACT, PSUM->SBUF fp16, scale fused)
  O'^T[d', q] = sum_j V'[j, d'] P^T[j, q]  (PE; V' = [V | ones] so row 64 = softmax sums)
  O[q, :] = O'^T[0:64, q].T / O'^T[64, q]  (PE transpose + DVE reciprocal/mult)

Layout tricks:
  - Q^T / K^T built by PE transposes of natural tiles, batch-PAIRED so batch A
    lands on SBUF partitions 0-63 and batch B on 64-127.  QK^T matmuls for the
    two batches then row-pack on the PE (disjoint row groups, concurrent).
  - S^T PSUM tiles are grouped 3-wide ([128, 1536]) so each ACT exp instruction
    amortizes its ~350-cycle overhead over 1536 elements.
"""

import sys
for _p in ("/opt/trn_rl_repo", "/root/.axon_site/_ro/trn_rl_repo"):
    if _p not in sys.path:
        sys.path.insert(0, _p)

import numpy as np

import concourse.bass as bass
import concourse.mybir as mybir
from concourse import bacc
from concourse.tile import TileContext
from concourse.masks import make_identity

F32 = mybir.dt.float32
F16 = mybir.dt.float16
P = 128

N_CORES = 8
B_FULL, N, D = 16, 2048, 64
B_LOC = B_FULL // N_CORES          # 2 batches per core
NT = N // P                        # 16 tiles of 128 along q and j
QB = 512                           # q-block (PSUM bank width in fp32)
NQB = N // QB                      # 4 q-blocks
GROUP = 3                          # S^T stream tiles per ACT exp instruction
N_STREAM = NQB * 2 * NT            # 128 S^T stream tiles per core
N_GROUPS = (N_STREAM + GROUP - 1) // GROUP

_nc_cache = None


def build():
    nc = bacc.Bacc(None, target_bir_lowering=False)
    q_hbm = nc.dram_tensor("queries", [B_LOC, N, D], F32, kind="ExternalInput")
    k_hbm = nc.dram_tensor("keys", [B_LOC, N, D], F32, kind="ExternalInput")
    v_hbm = nc.dram_tensor("values", [B_LOC, N, D], F32, kind="ExternalInput")
    o_hbm = nc.dram_tensor("out", [B_LOC, N, D], F32, kind="ExternalOutput")

    with TileContext(nc) as tc:
        with (
            tc.tile_pool(name="cst", bufs=1) as cst,
            tc.tile_pool(name="stage", bufs=2) as stage,
            tc.tile_pool(name="persist", bufs=1) as persist,
            tc.tile_pool(name="pt", bufs=24) as ptp,
            tc.tile_pool(name="otp", bufs=4) as otp,
            tc.tile_pool(name="ost", bufs=4) as ostp,
            tc.tile_pool(name="st", bufs=2, space="PSUM") as stp,
            tc.tile_pool(name="pv", bufs=2, space="PSUM") as pvp,
        ):
            ident = cst.tile([P, P], F16)
            make_identity(nc, ident)

            # ---- persistent SBUF buffers ----
            # Q^T / K^T, batch-paired: rows 0-63 batch A (d), 64-127 batch B.
            qt = persist.tile([P, N], F16, tag="qt")
            kt = persist.tile([P, N], F16, tag="kt")
            # V' = [V | ones]: [128 j, b, jt, 65] fp16
            v16 = persist.tile([P, B_LOC, NT, D + 1], F16, tag="v16")

            # ---- prep: load, cast, transpose ----
            # staging layout [128, t, (b d)]: per partition q, tile t, batch-major d
            q_res = [q_hbm[b, :, :].rearrange("(t p) d -> p t d", p=P)
                     for b in range(B_LOC)]
            k_res = [k_hbm[b, :, :].rearrange("(t p) d -> p t d", p=P)
                     for b in range(B_LOC)]

            # Loads: each dma_start costs ~650ns serial issue on the SP HWDGE
            # ring and the data phases serialize across the 16 SDMA engines, so
            # order chunks by when the pipeline needs them: K/Q tiles 0-3
            # first (first matmuls), then 4-7, then the rest, V before Q 8-15.
            st32s, st16s = {}, {}
            for name in ("k", "q"):
                st32s[name] = stage.tile([P, NT, B_LOC, D], F32, tag=f"{name}s32",
                                         name=f"{name}s32")
                st16s[name] = stage.tile([P, NT, B_LOC, D], F16, tag=f"{name}s16",
                                         name=f"{name}s16")
            vs32 = stage.tile([P, B_LOC, NT, D], F32, tag="vs32")

            def load_chunk(name, t0, t1, eng=None):
                re_aps = k_res if name == "k" else q_res
                cs = slice(t0, t1)
                for b in range(B_LOC):
                    (eng or nc.sync).dma_start(st32s[name][:, cs, b, :],
                                               re_aps[b][:, cs, :])
                nc.vector.tensor_copy(st16s[name][:, cs], st32s[name][:, cs])

            load_chunk("k", 0, 4)
            load_chunk("q", 0, 4)
            load_chunk("k", 4, 8)
            load_chunk("q", 4, 8)
            load_chunk("k", 8, NT)
            # V: [128, b, t, d] staging -> cast into v16[:, :, :, :64], ones col
            for b in range(B_LOC):
                v_re = v_hbm[b, :, :].rearrange("(t p) d -> p t d", p=P)
                nc.sync.dma_start(vs32[:, b], v_re)
            nc.vector.tensor_copy(v16[:, :, :, 0:D], vs32[:])
            nc.vector.memset(v16[:, :, :, D:D + 1], 1.0)
            load_chunk("q", 8, NT)

            def prep_transpose(name, t, dst):
                """PE-transpose natural tile t of q/k into dst[:, t*128...]."""
                tp_ps = pvp.tile([P, P], F16, tag="pv", name=f"tp_{name}{t}")
                nc.tensor.transpose(tp_ps[:], st16s[name][:, t], ident[:])
                nc.vector.tensor_copy(dst[:, t * P:(t + 1) * P], tp_ps[:])

            # K^T tiles j0-1 now; the rest are emitted lazily inside the qb0
            # loop (pool allocations are FIFO in emission order, so a transpose
            # emitted before its staging data lands would stall the slot ring).
            for t in range(2):
                prep_transpose("k", t, kt)

            # ---- main loop ----
            # stream of S^T tiles: s = qb*32 + 2*j + b
            st_tiles = {}      # group -> psum tile
            pt_tiles = {}      # group -> sbuf fp16 tile
            o_ps = {}          # b -> current PV accumulation psum tile

            # groups never span a q-block boundary: per qb there are 2*NT=32
            # stream tiles -> 10 groups of 3 + 1 group of 2.
            SPQ = 2 * NT                       # stream tiles per q-block
            GPQ = (SPQ + GROUP - 1) // GROUP   # groups per q-block

            def group_of(s):
                qb_, sl = divmod(s, SPQ)
                return qb_ * GPQ + sl // GROUP, sl % GROUP

            def tiles_in_group(g):
                return min(GROUP, SPQ - (g % GPQ) * GROUP)

            def maybe_exp(g):
                """emit exp for group g once all its stream tiles are written"""
                n_in_g = tiles_in_group(g)
                pt_t = ptp.tile([P, GROUP * QB], F16, tag="pt", name=f"ptg{g}")
                nc.scalar.activation(
                    pt_t[:, :n_in_g * QB], st_tiles[g][:, :n_in_g * QB],
                    mybir.ActivationFunctionType.Exp, scale=0.125,
                )
                pt_tiles[g] = pt_t

            # Q^T tiles for q-block 0
            QTPB = QB // P  # q-tiles per q-block
            for t in range(QTPB):
                prep_transpose("q", t, qt)

            for qb in range(NQB):
                qs = slice(qb * QB, (qb + 1) * QB)
                # PV trails QK by PV_LAG steps.  qb0's pv-pool FIFO is occupied
                # by the lazy K transposes until step 13, so its PV starts at
                # step 14; later blocks keep PV after the full QK loop (the
                # scheduler back-fills PE gaps with it).
                PV_LAG = 14 if qb == 0 else 4
                for step in range(NT + PV_LAG):
                    if step < NT:
                        j = step
                        if qb == 0 and j + 2 < NT:
                            prep_transpose("k", j + 2, kt)
                        # next q-block's Q^T transposes early (before this
                        # block's o_ps claims the pv slots at PV_LAG)
                        tp_q_step = 8 if qb == 0 else 0
                        if qb + 1 < NQB and tp_q_step <= step < tp_q_step + 2:
                            for i in range(2):
                                prep_transpose("q", (qb + 1) * QTPB + 2 * (step - tp_q_step) + i, qt)
                        for b in range(B_LOC):
                            s = qb * SPQ + 2 * j + b
                            g, slot = group_of(s)
                            if slot == 0:
                                st_tiles[g] = stp.tile([P, GROUP * QB], F32, tag="st", name=f"stg{g}")
                            rows = slice(b * D, (b + 1) * D)
                            nc.tensor.matmul(
                                st_tiles[g][:, slot * QB:(slot + 1) * QB],
                                kt[rows, j * P:(j + 1) * P],
                                qt[rows, qs],
                                start=True, stop=True,
                            )
                            if slot == tiles_in_group(g) - 1:
                                maybe_exp(g)
                    if step == PV_LAG:
                        for b in range(B_LOC):
                            o_ps[b] = pvp.tile([D + 1, QB], F32, tag="pv", name=f"opv{b}")
                    # PV: the two batch chains interleave per j so neither
                    # waits for the other to fully drain.
                    if step >= PV_LAG:
                        j = step - PV_LAG
                        for b in range(B_LOC):
                            s = qb * SPQ + 2 * j + b
                            g, slot = group_of(s)
                            nc.tensor.matmul(
                                o_ps[b][:],
                                v16[:, b, j, :],
                                pt_tiles[g][:, slot * QB:(slot + 1) * QB],
                                start=(j == 0), stop=(j == NT - 1),
                                skip_group_check=True,
                            )
                for b in range(B_LOC):
                    # drain O'^T -> fp16 staging; transpose all 4 q-tiles into
                    # ONE psum bank (fp16 4*65=260 elems); single reciprocal of
                    # the 4 sums columns; one broadcast multiply; one store.
                    ot_sb = otp.tile([D + 1, QB], F16, tag="ot", name=f"ot{b}")
                    nc.vector.tensor_copy(ot_sb[:], o_ps[b][:])
                    tp4 = pvp.tile([P, QTPB, D + 2], F16, tag="pv", name=f"tp4_{b}")
                    for t in range(QTPB):
                        nc.tensor.transpose(
                            tp4[:, t, 0:D + 1], ot_sb[:, t * P:(t + 1) * P],
                            ident[:D + 1, :D + 1],
                        )
                    o_out = ostp.tile([P, QTPB, D], F32, tag="oo", name=f"oo{b}")
                    recip4 = ostp.tile([P, QTPB, 1], F32, tag="recip", name=f"recip{b}")
                    nc.vector.reciprocal(recip4[:], tp4[:, :, D:D + 1])
                    nc.vector.tensor_tensor(
                        o_out[:], tp4[:, :, 0:D],
                        recip4[:].to_broadcast((P, QTPB, D)),
                        mybir.AluOpType.mult,
                    )
                    o_dst = o_hbm[b, qs, :].rearrange("(t p) d -> p t d", p=P)
                    nc.sync.dma_start(o_dst, o_out[:])

    nc.compile()
    return nc


def get_nc():
    global _nc_cache
    if _nc_cache is None:
        _nc_cache = build()
    return _nc_cache


def kernel(queries: np.ndarray, keys: np.ndarray, values: np.ndarray) -> np.ndarray:
    from concourse.bass_utils import run_bass_kernel_spmd

    queries = np.ascontiguousarray(np.asarray(queries, dtype=np.float32))
    keys = np.ascontiguousarray(np.asarray(keys, dtype=np.float32))
    values = np.ascontiguousarray(np.asarray(values, dtype=np.float32))

    nc = get_nc()
    in_maps = []
    for c in range(N_CORES):
        sl = slice(c * B_LOC, (c + 1) * B_LOC)
        in_maps.append({
            "queries": queries[sl],
            "keys": keys[sl],
            "values": values[sl],
        })
    res = run_bass_kernel_spmd(nc, in_maps, core_ids=list(range(N_CORES)))
    return np.concatenate([r["out"] for r in res.results], axis=0)


if __name__ == "__main__":
    rng = np.random.default_rng(0)
    q = rng.standard_normal((B_FULL, N, D), dtype=np.float32)
    k = rng.standard_normal((B_FULL, N, D), dtype=np.float32)
    v = rng.standard_normal((B_FULL, N, D), dtype=np.float32)
    o = kernel(queries=q, keys=k, values=v)
    s = q @ k.transpose(0, 2, 1) / np.sqrt(D)
    w = np.exp(s - s.max(-1, keepdims=True))
    w /= w.sum(-1, keepdims=True)
    ref = w @ v
    err = np.abs(o - ref).max() / np.abs(ref).max()
    print("rel err:", err)


# revision 45
# speedup vs baseline: 1.0200x; 1.0011x over previous
"""Trainium2 Bass kernel: batched dense attention.

Full inputs: queries/keys/values [16, 2048, 64] fp32.
Shards batch dim across 8 NeuronCores (2 batches per core).

Per-core algorithm (batches A, B local):
  S^T[j, q] = K[j, :] . Q[q, :]           (PE, fp16 operands, fp32 PSUM)
  P^T = exp(S^T / 8)                       (ACT, PSUM->SBUF fp16, scale fused)
  O'^T[d', q] = sum_j V'[j, d'] P^T[j, q]  (PE; V' = [V | ones] so row 64 = softmax sums)
  O[q, :] = O'^T[0:64, q].T / O'^T[64, q]  (PE transpose + DVE reciprocal/mult)

Layout tricks:
  - Q^T / K^T built by PE transposes of natural tiles, batch-PAIRED so batch A
    lands on SBUF partitions 0-63 and batch B on 64-127.  QK^T matmuls for the
    two batches then row-pack on the PE (disjoint row groups, concurrent).
  - S^T PSUM tiles are grouped 3-wide ([128, 1536]) so each ACT exp instruction
    amortizes its per-instruction overhead; the ACT engine is the bottleneck
    (~64us busy of ~84us modeled total per core), so the whole schedule is
    arranged to keep it fed: 2 rotating 3-bank S^T buffers, loads chunked by
    need, prep transposes and PV chains placed so the pv-PSUM pool's FIFO
    allocation order matches data arrival.
  - All matmul operands are fp16 (1 cycle/row on the PE, ~5e-4 relative error
    for this data); PSUM accumulation stays fp32.
  - The softmax denominators ride along as a 65th "ones" column of V, and the
    final transpose packs all four [65,128] tiles of a q-block into one PSUM
    bank, divided by a single broadcast multiply.
"""

import sys
for _p in ("/opt/trn_rl_repo", "/root/.axon_site/_ro/trn_rl_repo"):
    if _p not in sys.path:
        sys.path.insert(0, _p)

import numpy as np

import concourse.bass as bass
import concourse.mybir as mybir
from concourse import bacc
from concourse.tile import TileContext
from concourse.masks import make_identity

F32 = mybir.dt.float32
F16 = mybir.dt.float16
P = 128

N_CORES = 8
B_FULL, N, D = 16, 2048, 64
B_LOC = B_FULL // N_CORES          # 2 batches per core
NT = N // P                        # 16 tiles of 128 along q and j
QB = 512                           # q-block (PSUM bank width in fp32)
NQB = N // QB                      # 4 q-blocks
GROUP = 3                          # S^T stream tiles per ACT exp instruction
N_STREAM = NQB * 2 * NT            # 128 S^T stream tiles per core
N_GROUPS = (N_STREAM + GROUP - 1) // GROUP

_nc_cache = None


def build():
    nc = bacc.Bacc(None, target_bir_lowering=False)
    q_hbm = nc.dram_tensor("queries", [B_LOC, N, D], F32, kind="ExternalInput")
    k_hbm = nc.dram_tensor("keys", [B_LOC, N, D], F32, kind="ExternalInput")
    v_hbm = nc.dram_tensor("values", [B_LOC, N, D], F32, kind="ExternalInput")
    o_hbm = nc.dram_tensor("out", [B_LOC, N, D], F32, kind="ExternalOutput")

    with TileContext(nc) as tc:
        with (
            tc.tile_pool(name="cst", bufs=1) as cst,
            tc.tile_pool(name="stage", bufs=2) as stage,
            tc.tile_pool(name="persist", bufs=1) as persist,
            tc.tile_pool(name="pt", bufs=24) as ptp,
            tc.tile_pool(name="otp", bufs=4) as otp,
            tc.tile_pool(name="ost", bufs=4) as ostp,
            tc.tile_pool(name="st", bufs=2, space="PSUM") as stp,
            tc.tile_pool(name="pv", bufs=2, space="PSUM") as pvp,
        ):
            ident = cst.tile([P, P], F16)
            make_identity(nc, ident)

            # ---- persistent SBUF buffers ----
            # Q^T / K^T, batch-paired: rows 0-63 batch A (d), 64-127 batch B.
            qt = persist.tile([P, N], F16, tag="qt")
            kt = persist.tile([P, N], F16, tag="kt")
            # V' = [V | ones]: [128 j, b, jt, 65] fp16
            v16 = persist.tile([P, B_LOC, NT, D + 1], F16, tag="v16")

            # ---- prep: load, cast, transpose ----
            # staging layout [128, t, (b d)]: per partition q, tile t, batch-major d
            q_res = [q_hbm[b, :, :].rearrange("(t p) d -> p t d", p=P)
                     for b in range(B_LOC)]
            k_res = [k_hbm[b, :, :].rearrange("(t p) d -> p t d", p=P)
                     for b in range(B_LOC)]

            # Loads: each dma_start costs ~650ns serial issue on the SP HWDGE
            # ring and the data phases serialize across the 16 SDMA engines, so
            # order chunks by when the pipeline needs them: K/Q tiles 0-3
            # first (first matmuls), then 4-7, then the rest, V before Q 8-15.
            st32s, st16s = {}, {}
            for name in ("k", "q"):
                st32s[name] = stage.tile([P, NT, B_LOC, D], F32, tag=f"{name}s32",
                                         name=f"{name}s32")
                st16s[name] = stage.tile([P, NT, B_LOC, D], F16, tag=f"{name}s16",
                                         name=f"{name}s16")
            vs32 = stage.tile([P, B_LOC, NT, D], F32, tag="vs32")

            def load_chunk(name, t0, t1, eng=None):
                re_aps = k_res if name == "k" else q_res
                cs = slice(t0, t1)
                for b in range(B_LOC):
                    (eng or nc.sync).dma_start(st32s[name][:, cs, b, :],
                                               re_aps[b][:, cs, :])
                nc.vector.tensor_copy(st16s[name][:, cs], st32s[name][:, cs])

            load_chunk("k", 0, 4)
            load_chunk("q", 0, 4)
            load_chunk("k", 4, 8)
            load_chunk("q", 4, 8)
            load_chunk("k", 8, NT)
            # V: [128, b, t, d] staging -> cast into v16[:, :, :, :64], ones col
            for b in range(B_LOC):
                v_re = v_hbm[b, :, :].rearrange("(t p) d -> p t d", p=P)
                nc.sync.dma_start(vs32[:, b], v_re)
            nc.vector.tensor_copy(v16[:, :, :, 0:D], vs32[:])
            nc.vector.memset(v16[:, :, :, D:D + 1], 1.0)
            load_chunk("q", 8, NT)

            def prep_transpose(name, t, dst, pool=None):
                """PE-transpose natural tile t of q/k into dst[:, t*128...]."""
                pool = pool or pvp
                tp_ps = pool.tile([P, P], F16, tag="st" if pool is stp else "pv",
                                  name=f"tp_{name}{t}")
                nc.tensor.transpose(tp_ps[:], st16s[name][:, t], ident[:])
                nc.vector.tensor_copy(dst[:, t * P:(t + 1) * P], tp_ps[:])

            # K^T tiles j0-1 now; the rest are emitted lazily inside the qb0
            # loop (pool allocations are FIFO in emission order, so a transpose
            # emitted before its staging data lands would stall the slot ring).
            for t in range(2):
                prep_transpose("k", t, kt, pool=(stp if t % 2 else pvp))

            # ---- main loop ----
            # stream of S^T tiles: s = qb*32 + 2*j + b
            st_tiles = {}      # group -> psum tile
            pt_tiles = {}      # group -> sbuf fp16 tile
            o_ps = {}          # b -> current PV accumulation psum tile

            # groups never span a q-block boundary: per qb there are 2*NT=32
            # stream tiles -> 10 groups of 3 + 1 group of 2.
            SPQ = 2 * NT                       # stream tiles per q-block
            GPQ = (SPQ + GROUP - 1) // GROUP   # groups per q-block

            def group_of(s):
                qb_, sl = divmod(s, SPQ)
                return qb_ * GPQ + sl // GROUP, sl % GROUP

            def tiles_in_group(g):
                return min(GROUP, SPQ - (g % GPQ) * GROUP)

            def maybe_exp(g):
                """emit exp for group g once all its stream tiles are written"""
                n_in_g = tiles_in_group(g)
                pt_t = ptp.tile([P, GROUP * QB], F16, tag="pt", name=f"ptg{g}")
                nc.scalar.activation(
                    pt_t[:, :n_in_g * QB], st_tiles[g][:, :n_in_g * QB],
                    mybir.ActivationFunctionType.Exp, scale=0.125,
                )
                pt_tiles[g] = pt_t

            # Q^T tiles for q-block 0
            QTPB = QB // P  # q-tiles per q-block
            for t in range(QTPB):
                prep_transpose("q", t, qt, pool=(stp if t % 2 else pvp))

            for qb in range(NQB):
                qs = slice(qb * QB, (qb + 1) * QB)
                # PV trails QK by PV_LAG steps.  qb0's pv-pool FIFO is occupied
                # by the lazy K transposes until step 13, so its PV starts at
                # step 14; later blocks keep PV after the full QK loop (the
                # scheduler back-fills PE gaps with it).
                PV_LAG = 14 if qb == 0 else 4
                for step in range(NT + PV_LAG):
                    if step < NT:
                        j = step
                        if qb == 0 and j + 2 < NT:
                            prep_transpose("k", j + 2, kt)
                        # next q-block's Q^T transposes early (before this
                        # block's o_ps claims the pv slots at PV_LAG)
                        tp_q_step = 8 if qb == 0 else 0
                        if qb + 1 < NQB and tp_q_step <= step < tp_q_step + 2:
                            for i in range(2):
                                prep_transpose("q", (qb + 1) * QTPB + 2 * (step - tp_q_step) + i, qt)
                        for b in range(B_LOC):
                            s = qb * SPQ + 2 * j + b
                            g, slot = group_of(s)
                            if slot == 0:
                                st_tiles[g] = stp.tile([P, GROUP * QB], F32, tag="st", name=f"stg{g}")
                            rows = slice(b * D, (b + 1) * D)
                            nc.tensor.matmul(
                                st_tiles[g][:, slot * QB:(slot + 1) * QB],
                                kt[rows, j * P:(j + 1) * P],
                                qt[rows, qs],
                                start=True, stop=True,
                            )
                            if slot == tiles_in_group(g) - 1:
                                maybe_exp(g)
                    if step == PV_LAG:
                        for b in range(B_LOC):
                            o_ps[b] = pvp.tile([D + 1, QB], F32, tag="pv", name=f"opv{b}")
                    # PV: the two batch chains interleave per j so neither
                    # waits for the other to fully drain.
                    if step >= PV_LAG:
                        j = step - PV_LAG
                        for b in range(B_LOC):
                            s = qb * SPQ + 2 * j + b
                            g, slot = group_of(s)
                            nc.tensor.matmul(
                                o_ps[b][:],
                                v16[:, b, j, :],
                                pt_tiles[g][:, slot * QB:(slot + 1) * QB],
                                start=(j == 0), stop=(j == NT - 1),
                                skip_group_check=True,
                            )
                for b in range(B_LOC):
                    # drain O'^T -> fp16 staging; transpose all 4 q-tiles into
                    # ONE psum bank (fp16 4*65=260 elems); single reciprocal of
                    # the 4 sums columns; one broadcast multiply; one store.
                    ot_sb = otp.tile([D + 1, QB], F16, tag="ot", name=f"ot{b}")
                    nc.vector.tensor_copy(ot_sb[:], o_ps[b][:])
                    tp4 = pvp.tile([P, QTPB, D + 2], F16, tag="pv", name=f"tp4_{b}")
                    for t in range(QTPB):
                        nc.tensor.transpose(
                            tp4[:, t, 0:D + 1], ot_sb[:, t * P:(t + 1) * P],
                            ident[:D + 1, :D + 1],
                        )
                    o_out = ostp.tile([P, QTPB, D], F32, tag="oo", name=f"oo{b}")
                    recip4 = ostp.tile([P, QTPB, 1], F32, tag="recip", name=f"recip{b}")
                    nc.vector.reciprocal(recip4[:], tp4[:, :, D:D + 1])
                    nc.vector.tensor_tensor(
                        o_out[:], tp4[:, :, 0:D],
                        recip4[:].to_broadcast((P, QTPB, D)),
                        mybir.AluOpType.mult,
                    )
                    o_dst = o_hbm[b, qs, :].rearrange("(t p) d -> p t d", p=P)
                    nc.sync.dma_start(o_dst, o_out[:])

    nc.compile()
    return nc


def get_nc():
    global _nc_cache
    if _nc_cache is None:
        _nc_cache = build()
    return _nc_cache


def kernel(queries: np.ndarray, keys: np.ndarray, values: np.ndarray) -> np.ndarray:
    from concourse.bass_utils import run_bass_kernel_spmd

    queries = np.ascontiguousarray(np.asarray(queries, dtype=np.float32))
    keys = np.ascontiguousarray(np.asarray(keys, dtype=np.float32))
    values = np.ascontiguousarray(np.asarray(values, dtype=np.float32))

    nc = get_nc()
    in_maps = []
    for c in range(N_CORES):
        sl = slice(c * B_LOC, (c + 1) * B_LOC)
        in_maps.append({
            "queries": queries[sl],
            "keys": keys[sl],
            "values": values[sl],
        })
    res = run_bass_kernel_spmd(nc, in_maps, core_ids=list(range(N_CORES)))
    return np.concatenate([r["out"] for r in res.results], axis=0)


if __name__ == "__main__":
    rng = np.random.default_rng(0)
    q = rng.standard_normal((B_FULL, N, D), dtype=np.float32)
    k = rng.standard_normal((B_FULL, N, D), dtype=np.float32)
    v = rng.standard_normal((B_FULL, N, D), dtype=np.float32)
    o = kernel(queries=q, keys=k, values=v)
    s = q @ k.transpose(0, 2, 1) / np.sqrt(D)
    w = np.exp(s - s.max(-1, keepdims=True))
    w /= w.sum(-1, keepdims=True)
    ref = w @ v
    err = np.abs(o - ref).max() / np.abs(ref).max()
    print("rel err:", err)
